# revision 38
# baseline (speedup 1.0000x reference)
"""Soft-DTW layer (band-limited, gamma=1) as a Bass/Tile kernel on 8 TRN2 cores.

Problem: x [64, 512] f32, protos [32, 64] f32 -> out [64, 32, 1] f32
  out[b, f, 0] = softDTW(C[b,f]) / T, C[b,f][i,j] = (x[b,i]-protos[f,j])^2,
  Sakoe-Chiba band |i/511 - j/63| <= 0.2, out-of-band = BIG.

Algorithm (per (b,f) problem, exp-space):
  E'(i,j) = e^{a*i - D(i,j)} satisfies
    E'(i,j) = G(i,j) * (E'(i-1,j) + E'(i-1,j-1) + e^{-a} * E'(i,j-1))
  with G = e^{a - C} (0 outside band). Sweep columns j=0..63; each column's
  in-band rows live in window [8j-104, 8j+112) (216 rows). Per column:
  one shifted-add (STT), one multiply (STT), one mult/add linear-recurrence
  scan (tensor_tensor_scan) along the stored window. Every FB columns a
  per-problem rescale s=1/max keeps values in f32 range; log(s) is
  accumulated and added back at the end.

Sharding: data-parallel over batch. Core c handles b in [8c, 8c+8); its 256
(b,f) problems sit as 2 groups of 128 partitions:
  partition p, group g -> b = 8c + 4g + p//32, f = p%32.
"""

import numpy as np

import concourse.bass as bass
import concourse.bacc as bacc
import concourse.mybir as mybir
import concourse.tile as tile
from concourse.bass_utils import run_bass_kernel_spmd

T, K = 512, 64
NCORES = 8
L = 216          # column window length
CS = L * 2 + 8   # column buffer: [g0 216 | g1 216 | 8 zero pad]
BOFF = 104       # column j covers rows [8j-104, 8j+112)
GCOL = 2 * L     # per-column G stride (both groups)
XPAD = 104 + T + 112          # padded x row length (728)
XBIG = 1.0e4                  # pad value; (XBIG-p)^2 ~ 1e8 -> exp -> 0
FILLBIG = 1.0e12              # out-of-band fill for C before exp
A = 0.75                      # rescale slope per row
FB = 4                        # feedback (renorm) every FB columns
EA = float(np.exp(A))
ECA = float(np.exp(-A))
F32 = mybir.dt.float32


def _ap(t, offset, dims):
    """Custom free-dim access pattern on tile t: dims = [[step, count], ...]
    (element units), keeping the partition dim."""
    ap = t[:, 0:1].copy()
    ap.ap = ap.ap[:1] + [[int(s), int(n)] for s, n in dims]
    ap.offset = int(offset)
    return ap


def _dram_ap(t, offset, pdims, fdims):
    """Custom access pattern on DRAM tensor t with explicit partition dims
    (pdims) and free dims (fdims), [step, count] in elements."""
    ap = t[:, :].copy()
    dims = [[int(s), int(n)] for s, n in pdims] + [[int(s), int(n)] for s, n in fdims]
    ap.ap = ap.ap[:1] + dims          # rust setter needs a VecI64Pair seed
    ap.ap = ap.ap[1:]                 # then drop the original leading dim
    ap.offset = int(offset)
    return ap


def build_nc(dump=False):
    nc = bacc.Bacc("TRN2")
    xs = nc.dram_tensor("xs", [8, T], F32, kind="ExternalInput")
    pr = nc.dram_tensor("protos", [32, K], F32, kind="ExternalInput")
    out = nc.dram_tensor("out", [128, 2], F32, kind="ExternalOutput")
    if dump:
        gdump = nc.dram_tensor("gdump", [128, K * GCOL], F32, kind="ExternalOutput")
        coldump = nc.dram_tensor("coldump", [128, 16 * CS], F32, kind="ExternalOutput")
        mxdump = nc.dram_tensor("mxdump", [128, 32], F32, kind="ExternalOutput")
        exdump = nc.dram_tensor("exdump", [128, 8], F32, kind="ExternalOutput")
        lmdump = nc.dram_tensor("lmdump", [128, 32], F32, kind="ExternalOutput")

    with tile.TileContext(nc) as tc:
        with tc.tile_pool(name="main", bufs=1) as pool:
            x_all = pool.tile([128, 2 * XPAD], F32)   # padded x per problem/group
            prt = pool.tile([128, K], F32)            # protos row per problem
            G = pool.tile([128, K * GCOL], F32)       # banded G, layout (j, g, v)
            colA = pool.tile([128, CS], F32)
            colB = pool.tile([128, CS], F32)
            w = pool.tile([128, 2 * L], F32)
            b = pool.tile([128, 2 * L], F32)
            mxb = pool.tile([128, 32], F32)           # 16 feedback slots x 2
            s2 = pool.tile([128, 2], F32)
            se = pool.tile([128, 2], F32)
            lnmx = pool.tile([128, 32], F32)          # 16 slots x 2 groups
            ef = pool.tile([128, 2], F32)
            efe = pool.tile([128, 2], mybir.dt.int32)
            eff = pool.tile([128, 2], F32)
            efm = pool.tile([128, 2], mybir.dt.int32)
            lnmant = pool.tile([128, 2], F32)
            lnef = pool.tile([128, 2], F32)
            lnS = pool.tile([128, 2], F32)
            tt = pool.tile([128, 2], F32)
            osb = pool.tile([128, 2], F32)
            acon = pool.tile([128, 1], F32)           # bias const A for Exp
            scr = pool.tile([128, 8], F32)            # DVE pre-touch scratch
            exd = pool.tile([128, 8], F32, name="exd") if dump else None
            fD = pool.tile([128, 1], F32)             # engine handoff flags
            fA = pool.tile([128, 1], F32)
            fP = pool.tile([128, 1], F32)
            sA = pool.tile([128, 1], F32)             # touch destinations
            sP = pool.tile([128, 1], F32)
            sD = pool.tile([128, 1], F32)

            # ---- init ----
            # memsets on the vector engine: the consumers are DVE ops, so
            # program order covers them without semaphore waits (the HW has
            # very few wait slots per instruction)
            nc.vector.memset(x_all[:, :], XBIG)
            nc.vector.memset(colA[:, :], 0.0)
            nc.vector.memset(colB[:, :], 0.0)
            nc.vector.memset(lnmx[:, :], 0.0)
            # virtual-corner seed E'(-1,-1)=e^{-a} at row -1 of column -1
            # (column -1 window starts at row -112; row -1 -> pos 111)
            nc.vector.memset(colA[:, 111:112], ECA)
            nc.vector.memset(colA[:, L + 111:L + 112], ECA)
            nc.vector.memset(acon[:, :], A)

            # x: DRAM [8, 512] -> per-group replicated rows (partition p,
            # group g reads row 4g + p//32)
            src0 = xs[0:4, :].unsqueeze(1).broadcast_to([4, 32, T])
            nc.sync.dma_start(x_all[:, BOFF:BOFF + T], src0)
            src1 = xs[4:8, :].unsqueeze(1).broadcast_to([4, 32, T])
            nc.sync.dma_start(x_all[:, XPAD + BOFF:XPAD + BOFF + T], src1)
            # protos: DRAM [32, 64] -> partition p reads row p%32
            psrc = pr[:, :].unsqueeze(0).broadcast_to([4, 32, K])
            nc.sync.dma_start(prt[:, :], psrc)
            # The HW has very few sem-wait slots per instruction, so the
            # first compute op must not wait on 3 DMA queues itself. Touch
            # each DMA'd region with a 1-wait DVE copy; the DVE clock then
            # covers the DMAs, and the fence keeps the real ops after.
            nc.vector.tensor_copy(scr[:, 0:1], x_all[:, BOFF:BOFF + 1])
            nc.vector.tensor_copy(scr[:, 1:2], x_all[:, XPAD + BOFF:XPAD + BOFF + 1])
            nc.vector.tensor_copy(scr[:, 2:3], prt[:, 0:1])
            tc.no_sync_barrier()

            # ---- G precompute ----
            # diff(j,g,v) = x_all[g*XPAD + 8j + v] - protos[p%32, j]
            g4 = G[:, :].rearrange("p (j g v) -> p j g v", j=K, g=2, v=L)
            xap = _ap(x_all, 0, [[8, K], [XPAD, 2], [1, L]])
            pap = _ap(prt, 0, [[1, K], [0, 2], [0, L]])
            nc.vector.tensor_tensor(g4, xap, pap, op=mybir.AluOpType.subtract)
            # Cross-engine handoffs are relayed through tiny flag tiles:
            # producer stamps a flag (same-engine data dep, no sem), the
            # consumer touches the flag (exactly 1 sem wait). This keeps
            # every instruction within the HW's sem-wait slot budget and
            # avoids WAR hazards on the big in-place buffer.
            nc.vector.tensor_copy(fD[:, :], G[:, 0:1])        # DVE stamp
            nc.scalar.copy(sA[:, :], fD[:, :])                # ACT sees DVE
            # C = diff^2, then G = exp(A - C) (both in place on ACT)
            nc.scalar.activation(G[:, :], G[:, :], mybir.ActivationFunctionType.Square)
            nc.scalar.activation(G[:, :], G[:, :], mybir.ActivationFunctionType.Exp,
                                 bias=acon[:, :], scale=-1.0)
            nc.scalar.copy(fA[:, :], G[:, 0:1])               # ACT stamp
            nc.gpsimd.tensor_copy(sP[:, :], fA[:, :])         # pool sees ACT
            # band mask: zero G outside 0 <= -7j + 63v - 114 <= 12876
            nc.gpsimd.affine_select(
                g4, g4, pattern=[[-7, K], [0, 2], [63, L]], base=-114,
                compare_op=mybir.AluOpType.is_ge, fill=0.0, channel_multiplier=0)
            nc.gpsimd.affine_select(
                g4, g4, pattern=[[7, K], [0, 2], [-63, L]], base=114 + 12876,
                compare_op=mybir.AluOpType.is_ge, fill=0.0, channel_multiplier=0)
            nc.gpsimd.tensor_copy(fP[:, :], G[:, 0:1])        # pool stamp
            nc.vector.tensor_copy(sD[:, :], fP[:, :])         # DVE sees pool
            nc.vector.tensor_copy(scr[:, 5:6], fA[:, :])      # DVE sees ACT
            tc.no_sync_barrier()
            if dump:
                nc.sync.dma_start(gdump[:, :], G[:, :])

            # ---- column DP ----
            fb_pending = False
            fb_k = 0
            cprev, ccur = colA, colB
            for j in range(K):
                gcol = G[:, j * GCOL:(j + 1) * GCOL]
                nc.vector.scalar_tensor_tensor(
                    w[:, :], cprev[:, 7:7 + 2 * L], EA, cprev[:, 8:8 + 2 * L],
                    op0=mybir.AluOpType.mult, op1=mybir.AluOpType.add)
                if fb_pending:
                    for g in range(2):
                        sl = slice(g * L, (g + 1) * L)
                        nc.vector.scalar_tensor_tensor(
                            b[:, sl], w[:, sl], se[:, g:g + 1],
                            G[:, j * GCOL + g * L: j * GCOL + (g + 1) * L],
                            op0=mybir.AluOpType.mult, op1=mybir.AluOpType.mult)
                    fb_pending = False
                else:
                    nc.vector.scalar_tensor_tensor(
                        b[:, :], w[:, :], ECA, gcol,
                        op0=mybir.AluOpType.mult, op1=mybir.AluOpType.mult)
                nc.vector.tensor_tensor_scan(
                    ccur[:, 0:2 * L], gcol, b[:, :], 0.0,
                    op0=mybir.AluOpType.mult, op1=mybir.AluOpType.add)
                if (j + 1) % FB == 0 and j < K - 1:
                    mx = mxb[:, 2 * fb_k:2 * fb_k + 2]
                    nc.vector.tensor_reduce(
                        mx, ccur[:, 0:2 * L].rearrange("p (g v) -> p g v", g=2),
                        axis=mybir.AxisListType.X, op=mybir.AluOpType.max)
                    nc.vector.reciprocal(s2[:, :], mx)
                    nc.vector.tensor_scalar_mul(se[:, :], s2[:, :], ECA)
                    nc.scalar.activation(lnmx[:, 2 * fb_k:2 * fb_k + 2], mx,
                                         mybir.ActivationFunctionType.Ln)
                    fb_k += 1
                    fb_pending = True
                if dump and (j + 1) % 4 == 0:
                    kk = (j + 1) // 4 - 1
                    nc.sync.dma_start(coldump[:, kk * CS:(kk + 1) * CS], ccur[:, :])
                cprev, ccur = ccur, cprev

            last = cprev  # column 63 buffer
            if dump:
                nc.sync.dma_start(mxdump[:, :], mxb[:, :])
            # ---- extraction: D = a*511 - sum(lnmx) - ln(E'fin); out = D/512 ----
            nc.vector.tensor_copy(ef[:, 0:1], last[:, 111:112])
            nc.vector.tensor_copy(ef[:, 1:2], last[:, L + 111:L + 112])
            # ACT's Ln mishandles tiny args (E'fin can be ~1e-37), so do a
            # frexp-style log: ln(ef) = Ln(mantissa) + (exp - 127)*ln2.
            # (the -127*ln2 is folded into the final affine)
            eiv = ef[:, :].bitcast(mybir.dt.int32)
            nc.vector.tensor_scalar(efe[:, :], eiv, 23, None,
                                    op0=mybir.AluOpType.arith_shift_right)
            nc.vector.tensor_copy(eff[:, :], efe[:, :])   # int -> float value
            nc.vector.tensor_scalar(efm[:, :], eiv, 0x007FFFFF, 0x3F800000,
                                    op0=mybir.AluOpType.bitwise_and,
                                    op1=mybir.AluOpType.bitwise_or)
            nc.scalar.activation(lnmant[:, :], efm[:, :].bitcast(F32),
                                 mybir.ActivationFunctionType.Ln)
            nc.vector.scalar_tensor_tensor(
                lnef[:, :], eff[:, :], float(np.log(2.0)), lnmant[:, :],
                op0=mybir.AluOpType.mult, op1=mybir.AluOpType.add)
            nc.vector.tensor_reduce(
                lnS[:, :], lnmx[:, :].rearrange("p (k g) -> p g k", g=2),
                axis=mybir.AxisListType.X, op=mybir.AluOpType.add)
            nc.vector.tensor_tensor(tt[:, :], lnS[:, :], lnef[:, :],
                                    op=mybir.AluOpType.add)
            nc.scalar.activation(osb[:, :], tt[:, :],
                                 mybir.ActivationFunctionType.Copy,
                                 bias=float((A * (T - 1) + 127.0 * np.log(2.0)) / T),
                                 scale=float(-1.0 / T))
            nc.sync.dma_start(out[:, :], osb[:, :])
            if dump:
                nc.vector.tensor_copy(exd[:, 0:2], ef[:, :])
                nc.vector.tensor_copy(exd[:, 2:4], lnef[:, :])
                nc.vector.tensor_copy(exd[:, 4:6], lnS[:, :])
                nc.vector.tensor_copy(exd[:, 6:8], tt[:, :])
                nc.sync.dma_start(exdump[:, :], exd[:, :])
                nc.sync.dma_start(lmdump[:, :], lnmx[:, :])

    nc.compile()
    return nc


_NC = None


def _get_nc():
    global _NC
    if _NC is None:
        _NC = build_nc()
    return _NC


def kernel(x: np.ndarray, protos: np.ndarray) -> np.ndarray:
    x = np.ascontiguousarray(x, dtype=np.float32)
    protos = np.ascontiguousarray(protos, dtype=np.float32)
    nc = _get_nc()
    in_maps = [
        {"xs": x[8 * c: 8 * c + 8], "protos": protos} for c in range(NCORES)
    ]
    res = run_bass_kernel_spmd(nc, in_maps, core_ids=list(range(NCORES)))
    out = np.empty((64, 32, 1), dtype=np.float32)
    for c in range(NCORES):
        r = res.results[c]["out"]                 # [128, 2]
        blk = r.reshape(4, 32, 2).transpose(2, 0, 1)  # [g, bb, f]
        out[8 * c: 8 * c + 8, :, 0] = blk.reshape(8, 32)
    return out


if __name__ == "__main__":
    x = np.load("/root/problem/x.npy")
    protos = np.load("/root/problem/protos.npy")
    got = kernel(x, protos)
    D_true = np.load("/root/problem/D_true.npy").reshape(64, 32) / T
    rel = np.abs(got[:, :, 0] - D_true) / np.abs(D_true)
    print("rel err max", rel.max(), "mean", rel.mean())
